# revision 40
# baseline (speedup 1.0000x reference)
"""Distributed Trainium2 Bass kernel for nn_Attention_68736656605774.

Dense transformer self-attention block:
  qkv = x @ W_qkv + b_qkv ; RoPE(q, k) ; scores = q k^T/sqrt(dh) + mask + bias
  softmax ; a = P v ; out = a @ W_out + b_out

Sharding (8 cores): tensor-parallel over heads for qkv+attention (2 heads
per core, full batch), per-batch-half AllGather of the per-head attention
outputs (the first overlaps the second batch half's attention compute),
then column-parallel output projection (each core computes 128 of the 1024
output features; host concatenates).

Layout choices:
 - Everything head-side is feature-major ("transposed"): qT/kT are
   [feat, seq] so scores are computed directly transposed [Sk, Sq].  The
   kv-mask becomes a per-partition additive bias of the exp() activation,
   softmax needs no max-subtraction (logits are O(5)), and the softmax
   denominator comes for free from an all-ones column appended to v.
 - attn_bias is pre-transposed on host to [b, h, k, q] (bf16) and added
   to the scores in PSUM via PE identity matmuls.  Experiments that moved
   this add to the vector engine (multiplicative exp(bias)) were slower:
   with fewer than ~6 matmuls per score tile the tensor engine blocks on
   the exp pipeline every iteration and never holds its boosted clock
   p-state, which costs more than the identity matmuls save.
 - softmax normalization uses a_norm = a * exp(-ln(denom)); compilation
   is patched so the single activation table that holds both Exp and Ln
   serves every activation, eliminating per-head table reloads.
 - Collectives: batch 0 gathers whole (hidden under batch 1's attention);
   batch 1 gathers per query-half so the first half's output projection
   overlaps the final collective.  A collective trigger parks its issuing
   queue (gpsimd) until completion, so everything the attention loop
   needs rides the sync/scalar queues instead.
 - b_qkv / b_out are all-zero in this problem spec and are not applied.
"""

import sys

sys.path.insert(0, "/opt/trn_rl_repo")

import numpy as np
import ml_dtypes

import concourse.bass as bass
import concourse.mybir as mybir
import concourse.tile as tile
from concourse import bacc
from concourse.bass_utils import run_bass_kernel_spmd
from concourse.masks import make_identity

BF16 = mybir.dt.bfloat16
F32 = mybir.dt.float32
NPBF16 = ml_dtypes.bfloat16

NCORES = 8
B, S, D, H = 2, 2048, 1024, 16
DH = D // H  # 64
HPC = H // NCORES  # heads per core = 2
BS = B * S  # 4096
MAX_POS = 10000
NEG = -1e9
EXP = mybir.ActivationFunctionType.Exp
LN = mybir.ActivationFunctionType.Ln
ADD = mybir.AluOpType.add
MULT = mybir.AluOpType.mult

_compiled = None


def _build():
    nc = bacc.Bacc(None, num_devices=NCORES)

    xT_d = nc.declare_dram_parameter("xT", [8, 128, BS], BF16, isOutput=False)
    wq_d = nc.declare_dram_parameter("wq", [128, 8, 128], BF16, isOutput=False)
    wk_d = nc.declare_dram_parameter("wk", [128, 8, 128], BF16, isOutput=False)
    wv_d = nc.declare_dram_parameter("wv", [128, 8, 128], BF16, isOutput=False)
    wout_d = nc.declare_dram_parameter("wout", [128, 8, 128], BF16,
                                       isOutput=False)
    cosq_d = nc.declare_dram_parameter("cosq", [128, S], BF16, isOutput=False)
    sinq_d = nc.declare_dram_parameter("sinq", [128, S], BF16, isOutput=False)
    cosk_d = nc.declare_dram_parameter("cosk", [128, S], BF16, isOutput=False)
    sink_d = nc.declare_dram_parameter("sink", [128, S], BF16, isOutput=False)
    maskv_d = nc.declare_dram_parameter("maskv", [128, 32], F32, isOutput=False)
    bias_d = nc.declare_dram_parameter("bias", [B, HPC, S, S], BF16, isOutput=False)
    out_d = nc.declare_dram_parameter("out", [128, BS], F32, isOutput=True)

    with tile.TileContext(nc) as tc:
        with (
            tc.tile_pool(name="persist", bufs=1) as pp,
            tc.tile_pool(name="dram", bufs=1, space="DRAM") as dram,
        ):
            # ---------------- persistent SBUF tensors ----------------
            q_sb = pp.tile([128, BS], BF16, name="q_sb")
            k_sb = pp.tile([128, BS], BF16, name="k_sb")
            v_sb = pp.tile([128, 32, 130], BF16, name="v_sb")
            maskv = pp.tile([128, 32], F32, name="maskv")
            ones64 = pp.tile([1, 64], F32, name="ones64")
            ident = pp.tile([128, 128], BF16, name="ident")
            wout_sb = pp.tile([128, 8, 128], BF16, name="wout_sb")

            nc.vector.memset(ones64[:], 1.0)
            make_identity(nc, ident[:])

            # ---------------- phase 1: qkv projection + rope ----------------
            with (
                tc.tile_pool(name="ps1", bufs=8, space="PSUM") as ps1,
                tc.tile_pool(name="p1t", bufs=2) as p1t,
                tc.tile_pool(name="p1w", bufs=1) as p1w,
                tc.tile_pool(name="p1x", bufs=1) as p1x,
            ):
                xt_sb = p1x.tile([128, 8, BS], BF16, name="xt_sb")
                wq_sb = p1w.tile([128, 8, 128], BF16, name="wq_sb")
                wk_sb = p1w.tile([128, 8, 128], BF16, name="wk_sb")
                wv_sb = p1w.tile([128, 8, 128], BF16, name="wv_sb")
                cosq = p1w.tile([128, S], BF16, name="cosq")
                sinq = p1w.tile([128, S], BF16, name="sinq")
                cosk = p1w.tile([128, S], BF16, name="cosk")
                sink = p1w.tile([128, S], BF16, name="sink")
                # the kk-th chunk of xT is consumed ~3us after the
                # (kk-1)-th: split the 8 MB across two DMA queues, with all
                # the (small) stationary weights ahead of the bulky xT
                # chunks on sync, so the gemm never starves on either
                # operand.  Rope tables ride the otherwise-idle gpsimd
                # queue.
                for kk in range(0, 8, 2):
                    nc.scalar.dma_start(xt_sb[:, kk, :], xT_d[kk])
                nc.sync.dma_start(wq_sb[:], wq_d[:])
                for kk in range(1, 8, 2):
                    nc.sync.dma_start(xt_sb[:, kk, :], xT_d[kk])
                nc.sync.dma_start(wk_sb[:], wk_d[:])
                nc.sync.dma_start(wv_sb[:], wv_d[:])
                nc.sync.dma_start(maskv[:], maskv_d[:])
                nc.gpsimd.dma_start(cosq[:], cosq_d[:])
                nc.gpsimd.dma_start(sinq[:], sinq_d[:])
                nc.gpsimd.dma_start(cosk[:], cosk_d[:])
                nc.gpsimd.dma_start(sink[:], sink_d[:])

                qraw = p1w.tile([128, BS], BF16, name="qraw")
                kraw = p1w.tile([128, BS], BF16, name="kraw")
                vt_sb = p1w.tile([128, BS], BF16, name="vt_sb")

                # qT/kT/vT = W^T @ xT, feature-major [2*64, 4096];
                # kk-outer keeps the stationary operand loaded across the
                # 8 column chunks.  PSUM->SBUF copies ride on ACT, which
                # is otherwise idle in this phase.
                for w_sb, raw in ((wq_sb, qraw), (wk_sb, kraw), (wv_sb, vt_sb)):
                    pss = [
                        ps1.tile([128, 512], F32, name=f"ps_qk{n}", tag="ps1")
                        for n in range(8)
                    ]
                    for kk in range(8):
                        for n in range(8):
                            nc.tensor.matmul(
                                pss[n][:],
                                w_sb[:, kk, :],
                                xt_sb[:, kk, n * 512:(n + 1) * 512],
                                start=(kk == 0),
                                stop=(kk == 7),
                            )
                    for n in range(8):
                        nc.scalar.copy(
                            raw[:, n * 512:(n + 1) * 512], pss[n][:]
                        )

                # rope: q' = q*cos + swap32(q*sinswap), batch 0 first so
                # attention can begin while batch 1 is still rotating
                nc.vector.memset(v_sb[:, :, 64:65], 1.0)
                nc.vector.memset(v_sb[:, :, 129:130], 1.0)
                for b in range(B):
                    for raw, dst, ctab, stab in (
                        (qraw, q_sb, cosq, sinq),
                        (kraw, k_sb, cosk, sink),
                    ):
                        cols = slice(b * S, (b + 1) * S)
                        t = p1t.tile([128, S], BF16, name="rope_t", tag="rt")
                        m = p1t.tile([128, S], BF16, name="rope_m", tag="rm")
                        nc.vector.tensor_tensor(
                            t[:], raw[:, cols], ctab[:], MULT
                        )
                        # m[p] = raw[swap32(p)] * sinswap[swap32(p)]: shift
                        # partitions on the write side (both DVE read ports
                        # must share a base partition)
                        for blk in range(4):
                            p0 = blk * 32
                            sr = (blk ^ 1) * 32
                            nc.vector.tensor_tensor(
                                m[p0:p0 + 32, :],
                                raw[sr:sr + 32, cols],
                                stab[sr:sr + 32, :],
                                MULT,
                            )
                        nc.vector.tensor_tensor(
                            dst[:, cols], t[:], m[:], ADD
                        )
                    # v = transpose(vT) -> [seq, feat] tiles (ones columns
                    # at 64 / 129 for the softmax denominator)
                    for mt in range(b * 16, b * 16 + 16):
                        pst = ps1.tile([128, 128], BF16, name="ps_t",
                                       tag="ps1")
                        nc.tensor.transpose(
                            pst[:], vt_sb[:, mt * 128:(mt + 1) * 128],
                            ident[:]
                        )
                        nc.scalar.copy(
                            v_sb[:, mt, :].rearrange(
                                "p (h d) -> p h d", h=2
                            )[:, :, 0:64],
                            pst[:].rearrange("p (h d) -> p h d", h=2),
                        )

            # wout is only needed for the output projection; load it on
            # the quiet gpsimd queue
            nc.gpsimd.dma_start(wout_sb[:], wout_d[:])

            # ---------------- phase 2: attention ----------------
            p4a = tc.alloc_tile_pool(name="p4a", bufs=1)
            af_sb = p4a.tile([128, 8, BS], BF16, name="af_sb")
            # chunks: batch 0 gathers whole (its flight hides under
            # batch 1's attention); batch 1 gathers per query-half so the
            # first half's output projection overlaps the last collective
            ag_in = [
                dram.tile([128, S], BF16, name="ag_in0"),
                dram.tile([128, 1024], BF16, name="ag_in1a"),
                dram.tile([128, 1024], BF16, name="ag_in1b"),
            ]
            ag_out = [
                dram.tile([D, S], BF16, addr_space="Shared", name="ag_out0"),
                dram.tile([D, 1024], BF16, addr_space="Shared",
                          name="ag_out1a"),
                dram.tile([D, 1024], BF16, addr_space="Shared",
                          name="ag_out1b"),
            ]
            with (
                tc.tile_pool(name="ps_s", bufs=3, space="PSUM") as ps_sp,
                tc.tile_pool(name="ps_av", bufs=1, space="PSUM") as ps_avp,
                tc.tile_pool(name="p2t", bufs=8) as p2t,
                tc.tile_pool(name="p2s", bufs=8) as p2s,
                tc.tile_pool(name="p2n", bufs=2) as p2n,
            ):
                def emit_norm_b(state):
                    # part B of softmax normalize: broadcast -ln(denom) via
                    # PE, exponentiate, scale, ship to the allgather bounce
                    # buffer, and launch the batch collective when its last
                    # head lands
                    u_sb, ln_sb, bb, hroww, pww, fire_chunk = state
                    ps_bc = ps_sp.tile([64, 1024], F32, name="ps_bc", tag="s")
                    for j in range(2):
                        nc.tensor.matmul(
                            ps_bc[:, j * 512:(j + 1) * 512],
                            ones64[:],
                            ln_sb[:, j * 512:(j + 1) * 512],
                            start=True,
                            stop=True,
                        )
                    einv = p2n.tile([64, 1024], BF16, name="einv", tag="einv")
                    nc.scalar.activation(einv[:], ps_bc[:], EXP, scale=-1.0)
                    a_sb = p2n.tile([64, 1024], BF16, name="a_sb", tag="a")
                    nc.vector.tensor_tensor(
                        a_sb[:], u_sb[0:64, :], einv[:], MULT
                    )
                    if bb == 0:
                        nc.sync.dma_start(
                            ag_in[0][hroww, pww * 1024:(pww + 1) * 1024],
                            a_sb[:],
                        )
                    else:
                        nc.sync.dma_start(
                            ag_in[1 + pww][hroww, :], a_sb[:]
                        )
                    if fire_chunk is not None:
                        nc.gpsimd.collective_compute(
                            "AllGather",
                            mybir.AluOpType.bypass,
                            replica_groups=[list(range(NCORES))],
                            ins=[ag_in[fire_chunk].opt()],
                            outs=[ag_out[fire_chunk].opt()],
                        )

                pending_norm = None
                for b in range(B):
                    for pw in range(2):
                        q0 = b * S + pw * 1024
                        for h in range(HPC):
                            hrow = slice(h * 64, (h + 1) * 64)
                            vcols = slice(65 * h, 65 * h + 65)
                            ps_av = ps_avp.tile([65, 1024], F32,
                                                name="ps_av", tag="av")
                            pipe = []  # software pipeline: PV lags 2 tiles
                            for sk in range(16):
                                tg = b * 16 + sk
                                krows = slice(b * S + sk * 128,
                                              b * S + (sk + 1) * 128)
                                bias_sb = p2t.tile([128, 1024], BF16,
                                                   name="bias_sb", tag="bias")
                                nc.sync.dma_start(
                                    bias_sb[:],
                                    bias_d[b, h, sk * 128:(sk + 1) * 128,
                                           pw * 1024:(pw + 1) * 1024],
                                )
                                ps_s = ps_sp.tile([128, 1024], F32,
                                                  name="ps_s", tag="s")
                                for j in range(2):
                                    nc.tensor.matmul(
                                        ps_s[:, j * 512:(j + 1) * 512],
                                        k_sb[hrow, krows],
                                        q_sb[hrow, q0 + j * 512:
                                             q0 + (j + 1) * 512],
                                        start=True,
                                        stop=False,
                                    )
                                # bias via PE identity matmuls: keeps the
                                # tensor engine the clear pipeline
                                # bottleneck (it only holds the boosted
                                # clock when it never blocks on upstream
                                # semaphores) and keeps the exp dependency
                                # chain entirely on-PE
                                for j in range(2):
                                    nc.tensor.matmul(
                                        ps_s[:, j * 512:(j + 1) * 512],
                                        ident[:],
                                        bias_sb[:, j * 512:(j + 1) * 512],
                                        start=False,
                                        stop=True,
                                    )
                                exp_sb = p2s.tile([128, 1024], BF16,
                                                  name="exp_sb", tag="es")
                                nc.scalar.activation(
                                    exp_sb[:], ps_s[:], EXP,
                                    bias=maskv[:, tg:tg + 1], scale=1.0,
                                )
                                if sk == 2 and pending_norm is not None:
                                    emit_norm_b(pending_norm)
                                    pending_norm = None
                                if b == 1 and pw == 1 and h == 0 \
                                        and sk == 4:
                                    # prefetch batch 0's gathered heads on
                                    # gpsimd, emitted after the b1a
                                    # collective trigger: the queue is
                                    # parked there anyway, and a waiting
                                    # DMA must never sit ahead of the bias
                                    # stream (the sync queue runs ~8 tiles
                                    # ahead; one in-queue wait starves the
                                    # attention pipeline)
                                    for kk in range(8):
                                        nc.gpsimd.dma_start(
                                            af_sb[:, kk, 0:S],
                                            ag_out[0][kk * 128:
                                                      (kk + 1) * 128, :],
                                        )
                                pipe.append((tg, exp_sb))
                                if len(pipe) > 1:
                                    ptg, pp_sb = pipe.pop(0)
                                    for j in range(2):
                                        nc.tensor.matmul(
                                            ps_av[:, j * 512:(j + 1) * 512],
                                            v_sb[:, ptg, vcols],
                                            pp_sb[:, j * 512:(j + 1) * 512],
                                            start=(ptg % 16 == 0),
                                            stop=False,
                                        )
                            for di, (ptg, pp_sb) in enumerate(pipe):
                                last = di == len(pipe) - 1
                                for j in range(2):
                                    nc.tensor.matmul(
                                        ps_av[:, j * 512:(j + 1) * 512],
                                        v_sb[:, ptg, vcols],
                                        pp_sb[:, j * 512:(j + 1) * 512],
                                        start=False,
                                        stop=last,
                                    )
                            # normalize part A: ps_av -> SBUF (DVE) +
                            # ln(denom) (ACT); part B deferred to the next
                            # head's sk==2 so the PE never waits on it
                            u_sb = p2n.tile([65, 1024], F32, name="u_sb",
                                            tag="u")
                            nc.vector.tensor_scalar_mul(
                                u_sb[:], ps_av[:], 1.0
                            )
                            ln_sb = p2n.tile([1, 1024], F32, name="ln_sb",
                                             tag="ln")
                            nc.scalar.activation(ln_sb[:], u_sb[64:65, :], LN)
                            if h == HPC - 1 and (b == 1 or pw == 1):
                                fire = 0 if b == 0 else 1 + pw
                            else:
                                fire = None
                            pending_norm = (u_sb, ln_sb, b, hrow, pw, fire)
                if pending_norm is not None:
                    emit_norm_b(pending_norm)
                    pending_norm = None

            # ---------------- phase 4: output projection ----------------
            # column-parallel: this core computes output features
            # c*128..c*128+128 (its W_out column slice), transposed:
            # outT = Wc^T @ a_full^T.  Batch 0's chain only depends on the
            # first allgather (already landed), so it runs while the
            # second collective is still in flight; batch 1's matmuls
            # pipeline per-kk behind its af loads.
            with (
                tc.tile_pool(name="ps_o", bufs=8, space="PSUM") as ps_op,
                tc.tile_pool(name="p4t", bufs=4) as p4t,
            ):
                for c, (col0, src_ag) in enumerate(
                    ((0, None), (S, ag_out[1]), (S + 1024, ag_out[2]))
                ):
                    if src_ag is not None:
                        # b1a rides sync: the gpsimd queue is parked on the
                        # final collective's completion wait
                        ldq = nc.sync if c == 1 else nc.gpsimd
                        for kk in range(8):
                            ldq.dma_start(
                                af_sb[:, kk, col0:col0 + 1024],
                                src_ag[kk * 128:(kk + 1) * 128, :],
                            )
                    nsub = 4 if c == 0 else 2
                    ps_o = [
                        ps_op.tile([128, 512], F32, name=f"ps_o{c}{n}",
                                   tag="o")
                        for n in range(nsub)
                    ]
                    for kk in range(8):
                        for n in range(nsub):
                            nc.tensor.matmul(
                                ps_o[n][:],
                                wout_sb[:, kk, :],
                                af_sb[:, kk, col0 + n * 512:
                                      col0 + (n + 1) * 512],
                                start=(kk == 0),
                                stop=(kk == 7),
                            )
                    for n in range(nsub):
                        o_sb = p4t.tile([128, 512], F32, name="o_sb",
                                        tag="os")
                        nc.vector.tensor_scalar_mul(o_sb[:], ps_o[n][:], 1.0)
                        nc.scalar.dma_start(
                            out_d[:, col0 + n * 512:col0 + (n + 1) * 512],
                            o_sb[:],
                        )
            p4a.release()

    # compile with every activation served by the one table that holds
    # both Exp and Ln, so the per-head Ln never evicts the Exp table
    import concourse.bacc as bacc_mod
    _orig_gat = bacc_mod.get_activation_tables

    def _gat(arch):
        tabs = _orig_gat(arch)
        name = "natural_log_exp_and_others"
        if name in tabs:
            return {k: (v if k == name else set()) for k, v in tabs.items()}
        return tabs

    bacc_mod.get_activation_tables = _gat
    try:
        nc.compile()
    finally:
        bacc_mod.get_activation_tables = _orig_gat
    return nc


def _rope_tables():
    scales = 1.0 / (MAX_POS ** (np.arange(0, DH, 2, dtype=np.float32) / DH))
    freqs = np.outer(np.arange(S, dtype=np.float32), scales)  # [S, 32]
    cos = np.cos(freqs).T  # [32, S]
    sin = np.sin(freqs).T
    cos_dup = np.concatenate([cos, cos], axis=0)  # [64, S]
    sinswap = np.concatenate([sin, -sin], axis=0)  # [64, S]
    cos_t = np.concatenate([cos_dup, cos_dup], axis=0)  # [128, S] (2 heads)
    sin_t = np.concatenate([sinswap, sinswap], axis=0)
    return cos_t, sin_t


def _prep_inputs(x, kv_mask, attn_bias, W_qkv, b_qkv, W_out, b_out):
    scale = 1.0 / np.sqrt(DH)
    xT = np.ascontiguousarray(
        x.reshape(BS, D).T.astype(NPBF16)
    ).reshape(8, 128, BS)
    cos_t, sin_t = _rope_tables()
    cosq = (cos_t * scale).astype(NPBF16)
    sinq = (sin_t * scale).astype(NPBF16)
    cosk = cos_t.astype(NPBF16)
    sink = sin_t.astype(NPBF16)
    # mask vector [128, 32]: col = b*16 + sk_tile, row = position within tile
    mv = np.where(kv_mask, 0.0, NEG).astype(np.float32)  # [B, S]
    maskv = np.ascontiguousarray(
        mv.reshape(B, 16, 128).transpose(2, 0, 1).reshape(128, 32)
    )
    # bias: [b, q, k, h] -> [b, h, k, q] (bf16)
    bias_t = attn_bias.transpose(0, 3, 2, 1).astype(NPBF16)

    in_maps = []
    for c in range(NCORES):
        h0 = HPC * c
        def pmajor(w):
            # [1024, 128] -> [128 partitions, 8 kk-chunks, 128]
            return np.ascontiguousarray(
                w.astype(NPBF16).reshape(8, 128, 128).transpose(1, 0, 2)
            )

        wq = pmajor(W_qkv[:, h0 * DH:h0 * DH + 128])
        wk = pmajor(W_qkv[:, D + h0 * DH:D + h0 * DH + 128])
        wv = pmajor(W_qkv[:, 2 * D + h0 * DH:2 * D + h0 * DH + 128])
        wout = pmajor(W_out[:, c * 128:(c + 1) * 128])
        bias_c = np.ascontiguousarray(bias_t[:, h0:h0 + HPC])
        in_maps.append({
            "xT": xT, "wq": wq, "wk": wk, "wv": wv, "wout": wout,
            "cosq": cosq, "sinq": sinq, "cosk": cosk, "sink": sink,
            "maskv": maskv, "bias": bias_c,
        })
    return in_maps


def _run(inputs, trace=False):
    global _compiled
    if _compiled is None:
        _compiled = _build()
    in_maps = _prep_inputs(**inputs)
    res = run_bass_kernel_spmd(
        _compiled, in_maps, list(range(NCORES)), trace=trace
    )
    # each core returns outT [128, 4096]; transpose and concat on features
    cols = [res.results[c]["out"].T for c in range(NCORES)]
    out = np.concatenate(cols, axis=1).reshape(B, S, D)
    return out, res


def kernel(**inputs):
    out, _ = _run(inputs, trace=False)
    return out


# revision 41
# speedup vs baseline: 1.0449x; 1.0449x over previous
"""Distributed Trainium2 Bass kernel for nn_Attention_68736656605774.

Dense transformer self-attention block:
  qkv = x @ W_qkv + b_qkv ; RoPE(q, k) ; scores = q k^T/sqrt(dh) + mask + bias
  softmax ; a = P v ; out = a @ W_out + b_out

Sharding (8 cores): tensor-parallel over heads for qkv+attention (2 heads
per core, full batch), per-batch-half AllGather of the per-head attention
outputs (the first overlaps the second batch half's attention compute),
then column-parallel output projection (each core computes 128 of the 1024
output features; host concatenates).

Layout choices:
 - Everything head-side is feature-major ("transposed"): qT/kT are
   [feat, seq] so scores are computed directly transposed [Sk, Sq].  The
   kv-mask becomes a per-partition additive bias of the exp() activation,
   softmax needs no max-subtraction (logits are O(5)), and the softmax
   denominator comes for free from an all-ones column appended to v.
 - attn_bias is pre-transposed on host to [b, h, k, q] (bf16) and added
   to the scores in PSUM via PE identity matmuls.  Experiments that moved
   this add to the vector engine (multiplicative exp(bias)) were slower:
   with fewer than ~6 matmuls per score tile the tensor engine blocks on
   the exp pipeline every iteration and never holds its boosted clock
   p-state, which costs more than the identity matmuls save.
 - softmax normalization uses a_norm = a * exp(-ln(denom)); compilation
   is patched so the single activation table that holds both Exp and Ln
   serves every activation, eliminating per-head table reloads.
 - Collectives: batch 0 gathers whole (hidden under batch 1's attention);
   batch 1 gathers per query-half so the first half's output projection
   overlaps the final collective.  A collective trigger parks its issuing
   queue (gpsimd) until completion, so everything the attention loop
   needs rides the sync/scalar queues instead.
 - b_qkv / b_out are all-zero in this problem spec and are not applied.
"""

import sys

sys.path.insert(0, "/opt/trn_rl_repo")

import numpy as np
import ml_dtypes

import concourse.bass as bass
import concourse.mybir as mybir
import concourse.tile as tile
from concourse import bacc
from concourse.bass_utils import run_bass_kernel_spmd
from concourse.masks import make_identity

BF16 = mybir.dt.bfloat16
F32 = mybir.dt.float32
NPBF16 = ml_dtypes.bfloat16

NCORES = 8
B, S, D, H = 2, 2048, 1024, 16
DH = D // H  # 64
HPC = H // NCORES  # heads per core = 2
BS = B * S  # 4096
MAX_POS = 10000
NEG = -1e9
EXP = mybir.ActivationFunctionType.Exp
LN = mybir.ActivationFunctionType.Ln
ADD = mybir.AluOpType.add
MULT = mybir.AluOpType.mult

_compiled = None


def _build():
    nc = bacc.Bacc(None, num_devices=NCORES)

    xT_d = nc.declare_dram_parameter("xT", [8, 128, BS], BF16, isOutput=False)
    wq_d = nc.declare_dram_parameter("wq", [128, 8, 128], BF16, isOutput=False)
    wk_d = nc.declare_dram_parameter("wk", [128, 8, 128], BF16, isOutput=False)
    wv_d = nc.declare_dram_parameter("wv", [128, 8, 128], BF16, isOutput=False)
    wout_d = nc.declare_dram_parameter("wout", [128, 8, 128], BF16,
                                       isOutput=False)
    cosq_d = nc.declare_dram_parameter("cosq", [128, S], BF16, isOutput=False)
    sinq_d = nc.declare_dram_parameter("sinq", [128, S], BF16, isOutput=False)
    cosk_d = nc.declare_dram_parameter("cosk", [128, S], BF16, isOutput=False)
    sink_d = nc.declare_dram_parameter("sink", [128, S], BF16, isOutput=False)
    maskv_d = nc.declare_dram_parameter("maskv", [128, 32], F32, isOutput=False)
    bias_d = nc.declare_dram_parameter("bias", [B, HPC, S, S], BF16, isOutput=False)
    out_d = nc.declare_dram_parameter("out", [128, BS], F32, isOutput=True)

    with tile.TileContext(nc) as tc:
        with (
            tc.tile_pool(name="persist", bufs=1) as pp,
            tc.tile_pool(name="dram", bufs=1, space="DRAM") as dram,
        ):
            # ---------------- persistent SBUF tensors ----------------
            q_sb = pp.tile([128, BS], BF16, name="q_sb")
            k_sb = pp.tile([128, BS], BF16, name="k_sb")
            v_sb = pp.tile([128, 32, 130], BF16, name="v_sb")
            maskv = pp.tile([128, 32], F32, name="maskv")
            ones64 = pp.tile([1, 64], F32, name="ones64")
            ident = pp.tile([128, 128], BF16, name="ident")
            wout_sb = pp.tile([128, 8, 128], BF16, name="wout_sb")

            nc.vector.memset(ones64[:], 1.0)
            make_identity(nc, ident[:])
            # touch Exp once so the activation-table load lands here, in
            # the DMA shadow, instead of at the first attention tile
            scr1 = pp.tile([1, 1], F32, name="scr1")
            nc.scalar.activation(scr1[:], ones64[:, 0:1], EXP)

            # ---------------- phase 1: qkv projection + rope ----------------
            with (
                tc.tile_pool(name="ps1", bufs=8, space="PSUM") as ps1,
                tc.tile_pool(name="p1t", bufs=2) as p1t,
                tc.tile_pool(name="p1w", bufs=1) as p1w,
                tc.tile_pool(name="p1x", bufs=1) as p1x,
            ):
                xt_sb = p1x.tile([128, 8, BS], BF16, name="xt_sb")
                wq_sb = p1w.tile([128, 8, 128], BF16, name="wq_sb")
                wk_sb = p1w.tile([128, 8, 128], BF16, name="wk_sb")
                wv_sb = p1w.tile([128, 8, 128], BF16, name="wv_sb")
                cosq = p1w.tile([128, S], BF16, name="cosq")
                sinq = p1w.tile([128, S], BF16, name="sinq")
                cosk = p1w.tile([128, S], BF16, name="cosk")
                sink = p1w.tile([128, S], BF16, name="sink")
                # the kk-th chunk of xT is consumed ~3us after the
                # (kk-1)-th: split the 8 MB across two DMA queues, with all
                # the (small) stationary weights ahead of the bulky xT
                # chunks on sync, so the gemm never starves on either
                # operand.  Rope tables ride the otherwise-idle gpsimd
                # queue.
                for kk in range(0, 8, 2):
                    nc.scalar.dma_start(xt_sb[:, kk, :], xT_d[kk])
                nc.sync.dma_start(wq_sb[:], wq_d[:])
                for kk in range(1, 8, 2):
                    nc.sync.dma_start(xt_sb[:, kk, :], xT_d[kk])
                nc.sync.dma_start(wk_sb[:], wk_d[:])
                nc.sync.dma_start(wv_sb[:], wv_d[:])
                nc.sync.dma_start(maskv[:], maskv_d[:])
                nc.gpsimd.dma_start(cosq[:], cosq_d[:])
                nc.gpsimd.dma_start(sinq[:], sinq_d[:])
                nc.gpsimd.dma_start(cosk[:], cosk_d[:])
                nc.gpsimd.dma_start(sink[:], sink_d[:])

                qraw = p1w.tile([128, BS], BF16, name="qraw")
                kraw = p1w.tile([128, BS], BF16, name="kraw")
                vt_sb = p1w.tile([128, BS], BF16, name="vt_sb")

                # qT/kT/vT = W^T @ xT, feature-major [2*64, 4096];
                # kk-outer keeps the stationary operand loaded across the
                # 8 column chunks.  PSUM->SBUF copies ride on ACT, which
                # is otherwise idle in this phase.
                for w_sb, raw in ((wq_sb, qraw), (wk_sb, kraw), (wv_sb, vt_sb)):
                    pss = [
                        ps1.tile([128, 512], F32, name=f"ps_qk{n}", tag="ps1")
                        for n in range(8)
                    ]
                    for kk in range(8):
                        for n in range(8):
                            nc.tensor.matmul(
                                pss[n][:],
                                w_sb[:, kk, :],
                                xt_sb[:, kk, n * 512:(n + 1) * 512],
                                start=(kk == 0),
                                stop=(kk == 7),
                            )
                    for n in range(8):
                        nc.scalar.copy(
                            raw[:, n * 512:(n + 1) * 512], pss[n][:]
                        )

                # rope: q' = q*cos + swap32(q*sinswap), batch 0 first so
                # attention can begin while batch 1 is still rotating
                nc.vector.memset(v_sb[:, :, 64:65], 1.0)
                nc.vector.memset(v_sb[:, :, 129:130], 1.0)
                for b in range(B):
                    for raw, dst, ctab, stab in (
                        (qraw, q_sb, cosq, sinq),
                        (kraw, k_sb, cosk, sink),
                    ):
                        cols = slice(b * S, (b + 1) * S)
                        t = p1t.tile([128, S], BF16, name="rope_t", tag="rt")
                        m = p1t.tile([128, S], BF16, name="rope_m", tag="rm")
                        nc.vector.tensor_tensor(
                            t[:], raw[:, cols], ctab[:], MULT
                        )
                        # m[p] = raw[swap32(p)] * sinswap[swap32(p)]: shift
                        # partitions on the write side (both DVE read ports
                        # must share a base partition)
                        for blk in range(4):
                            p0 = blk * 32
                            sr = (blk ^ 1) * 32
                            nc.vector.tensor_tensor(
                                m[p0:p0 + 32, :],
                                raw[sr:sr + 32, cols],
                                stab[sr:sr + 32, :],
                                MULT,
                            )
                        nc.vector.tensor_tensor(
                            dst[:, cols], t[:], m[:], ADD
                        )
                    # v = transpose(vT) -> [seq, feat] tiles (ones columns
                    # at 64 / 129 for the softmax denominator)
                    for mt in range(b * 16, b * 16 + 16):
                        pst = ps1.tile([128, 128], BF16, name="ps_t",
                                       tag="ps1")
                        nc.tensor.transpose(
                            pst[:], vt_sb[:, mt * 128:(mt + 1) * 128],
                            ident[:]
                        )
                        nc.scalar.copy(
                            v_sb[:, mt, :].rearrange(
                                "p (h d) -> p h d", h=2
                            )[:, :, 0:64],
                            pst[:].rearrange("p (h d) -> p h d", h=2),
                        )

            # wout is only needed for the output projection; load it on
            # the quiet gpsimd queue
            nc.gpsimd.dma_start(wout_sb[:], wout_d[:])

            # ---------------- phase 2: attention ----------------
            p4a = tc.alloc_tile_pool(name="p4a", bufs=1)
            af_sb = p4a.tile([128, 8, BS], BF16, name="af_sb")
            # chunks: batch 0 gathers whole (its flight hides under
            # batch 1's attention); batch 1 gathers per query-half so the
            # first half's output projection overlaps the last collective
            ag_in = [
                dram.tile([128, S], BF16, name="ag_in0"),
                dram.tile([128, 1024], BF16, name="ag_in1a"),
                dram.tile([128, 1024], BF16, name="ag_in1b"),
            ]
            ag_out = [
                dram.tile([D, S], BF16, addr_space="Shared", name="ag_out0"),
                dram.tile([D, 1024], BF16, addr_space="Shared",
                          name="ag_out1a"),
                dram.tile([D, 1024], BF16, addr_space="Shared",
                          name="ag_out1b"),
            ]
            with (
                tc.tile_pool(name="ps_s", bufs=3, space="PSUM") as ps_sp,
                tc.tile_pool(name="ps_av", bufs=1, space="PSUM") as ps_avp,
                tc.tile_pool(name="p2t", bufs=8) as p2t,
                tc.tile_pool(name="p2s", bufs=8) as p2s,
                tc.tile_pool(name="p2n", bufs=2) as p2n,
            ):
                def emit_norm_b(state):
                    # part B of softmax normalize: broadcast -ln(denom) via
                    # PE, exponentiate, scale, ship to the allgather bounce
                    # buffer, and launch the batch collective when its last
                    # head lands
                    u_sb, ln_sb, bb, hroww, pww, fire_chunk = state
                    ps_bc = ps_sp.tile([64, 1024], F32, name="ps_bc", tag="s")
                    for j in range(2):
                        nc.tensor.matmul(
                            ps_bc[:, j * 512:(j + 1) * 512],
                            ones64[:],
                            ln_sb[:, j * 512:(j + 1) * 512],
                            start=True,
                            stop=True,
                        )
                    einv = p2n.tile([64, 1024], BF16, name="einv", tag="einv")
                    nc.scalar.activation(einv[:], ps_bc[:], EXP, scale=-1.0)
                    a_sb = p2n.tile([64, 1024], BF16, name="a_sb", tag="a")
                    nc.vector.tensor_tensor(
                        a_sb[:], u_sb[0:64, :], einv[:], MULT
                    )
                    if bb == 0:
                        nc.sync.dma_start(
                            ag_in[0][hroww, pww * 1024:(pww + 1) * 1024],
                            a_sb[:],
                        )
                    else:
                        nc.sync.dma_start(
                            ag_in[1 + pww][hroww, :], a_sb[:]
                        )
                    if fire_chunk is not None:
                        nc.gpsimd.collective_compute(
                            "AllGather",
                            mybir.AluOpType.bypass,
                            replica_groups=[list(range(NCORES))],
                            ins=[ag_in[fire_chunk].opt()],
                            outs=[ag_out[fire_chunk].opt()],
                        )

                pending_norm = None
                for b in range(B):
                    for pw in range(2):
                        q0 = b * S + pw * 1024
                        for h in range(HPC):
                            hrow = slice(h * 64, (h + 1) * 64)
                            vcols = slice(65 * h, 65 * h + 65)
                            ps_av = ps_avp.tile([65, 1024], F32,
                                                name="ps_av", tag="av")
                            pipe = []  # software pipeline: PV lags 2 tiles
                            for sk in range(16):
                                tg = b * 16 + sk
                                krows = slice(b * S + sk * 128,
                                              b * S + (sk + 1) * 128)
                                bias_sb = p2t.tile([128, 1024], BF16,
                                                   name="bias_sb", tag="bias")
                                nc.sync.dma_start(
                                    bias_sb[:],
                                    bias_d[b, h, sk * 128:(sk + 1) * 128,
                                           pw * 1024:(pw + 1) * 1024],
                                )
                                ps_s = ps_sp.tile([128, 1024], F32,
                                                  name="ps_s", tag="s")
                                for j in range(2):
                                    nc.tensor.matmul(
                                        ps_s[:, j * 512:(j + 1) * 512],
                                        k_sb[hrow, krows],
                                        q_sb[hrow, q0 + j * 512:
                                             q0 + (j + 1) * 512],
                                        start=True,
                                        stop=False,
                                    )
                                # bias via PE identity matmuls: keeps the
                                # tensor engine the clear pipeline
                                # bottleneck (it only holds the boosted
                                # clock when it never blocks on upstream
                                # semaphores) and keeps the exp dependency
                                # chain entirely on-PE
                                for j in range(2):
                                    nc.tensor.matmul(
                                        ps_s[:, j * 512:(j + 1) * 512],
                                        ident[:],
                                        bias_sb[:, j * 512:(j + 1) * 512],
                                        start=False,
                                        stop=True,
                                    )
                                exp_sb = p2s.tile([128, 1024], BF16,
                                                  name="exp_sb", tag="es")
                                nc.scalar.activation(
                                    exp_sb[:], ps_s[:], EXP,
                                    bias=maskv[:, tg:tg + 1], scale=1.0,
                                )
                                if sk == 2 and pending_norm is not None:
                                    emit_norm_b(pending_norm)
                                    pending_norm = None
                                if b == 1 and pw == 1 and h == 0 \
                                        and sk == 4:
                                    # prefetch batch 0's gathered heads on
                                    # gpsimd, emitted after the b1a
                                    # collective trigger: the queue is
                                    # parked there anyway, and a waiting
                                    # DMA must never sit ahead of the bias
                                    # stream (the sync queue runs ~8 tiles
                                    # ahead; one in-queue wait starves the
                                    # attention pipeline)
                                    for kk in range(8):
                                        nc.gpsimd.dma_start(
                                            af_sb[:, kk, 0:S],
                                            ag_out[0][kk * 128:
                                                      (kk + 1) * 128, :],
                                        )
                                pipe.append((tg, exp_sb))
                                if len(pipe) > 1:
                                    ptg, pp_sb = pipe.pop(0)
                                    for j in range(2):
                                        nc.tensor.matmul(
                                            ps_av[:, j * 512:(j + 1) * 512],
                                            v_sb[:, ptg, vcols],
                                            pp_sb[:, j * 512:(j + 1) * 512],
                                            start=(ptg % 16 == 0),
                                            stop=False,
                                        )
                            for di, (ptg, pp_sb) in enumerate(pipe):
                                last = di == len(pipe) - 1
                                for j in range(2):
                                    nc.tensor.matmul(
                                        ps_av[:, j * 512:(j + 1) * 512],
                                        v_sb[:, ptg, vcols],
                                        pp_sb[:, j * 512:(j + 1) * 512],
                                        start=False,
                                        stop=last,
                                    )
                            # normalize part A: ps_av -> SBUF (DVE) +
                            # ln(denom) (ACT); part B deferred to the next
                            # head's sk==2 so the PE never waits on it
                            u_sb = p2n.tile([65, 1024], F32, name="u_sb",
                                            tag="u")
                            nc.vector.tensor_scalar_mul(
                                u_sb[:], ps_av[:], 1.0
                            )
                            ln_sb = p2n.tile([1, 1024], F32, name="ln_sb",
                                             tag="ln")
                            nc.scalar.activation(ln_sb[:], u_sb[64:65, :], LN)
                            if h == HPC - 1 and (b == 1 or pw == 1):
                                fire = 0 if b == 0 else 1 + pw
                            else:
                                fire = None
                            pending_norm = (u_sb, ln_sb, b, hrow, pw, fire)
                if pending_norm is not None:
                    emit_norm_b(pending_norm)
                    pending_norm = None

            # ---------------- phase 4: output projection ----------------
            # column-parallel: this core computes output features
            # c*128..c*128+128 (its W_out column slice), transposed:
            # outT = Wc^T @ a_full^T.  Batch 0's chain only depends on the
            # first allgather (already landed), so it runs while the
            # second collective is still in flight; batch 1's matmuls
            # pipeline per-kk behind its af loads.
            with (
                tc.tile_pool(name="ps_o", bufs=8, space="PSUM") as ps_op,
                tc.tile_pool(name="p4t", bufs=4) as p4t,
            ):
                for c, (col0, src_ag) in enumerate(
                    ((0, None), (S, ag_out[1]), (S + 1024, ag_out[2]))
                ):
                    if src_ag is not None:
                        # b1a rides sync: the gpsimd queue is parked on the
                        # final collective's completion wait
                        ldq = nc.sync if c == 1 else nc.gpsimd
                        for kk in range(8):
                            ldq.dma_start(
                                af_sb[:, kk, col0:col0 + 1024],
                                src_ag[kk * 128:(kk + 1) * 128, :],
                            )
                    nsub = 4 if c == 0 else 2
                    ps_o = [
                        ps_op.tile([128, 512], F32, name=f"ps_o{c}{n}",
                                   tag="o")
                        for n in range(nsub)
                    ]
                    for kk in range(8):
                        for n in range(nsub):
                            nc.tensor.matmul(
                                ps_o[n][:],
                                wout_sb[:, kk, :],
                                af_sb[:, kk, col0 + n * 512:
                                      col0 + (n + 1) * 512],
                                start=(kk == 0),
                                stop=(kk == 7),
                            )
                    for n in range(nsub):
                        o_sb = p4t.tile([128, 512], F32, name="o_sb",
                                        tag="os")
                        nc.vector.tensor_scalar_mul(o_sb[:], ps_o[n][:], 1.0)
                        nc.scalar.dma_start(
                            out_d[:, col0 + n * 512:col0 + (n + 1) * 512],
                            o_sb[:],
                        )
            p4a.release()

    # compile with every activation served by the one table that holds
    # both Exp and Ln, so the per-head Ln never evicts the Exp table
    import concourse.bacc as bacc_mod
    _orig_gat = bacc_mod.get_activation_tables

    def _gat(arch):
        tabs = _orig_gat(arch)
        name = "natural_log_exp_and_others"
        if name in tabs:
            return {k: (v if k == name else set()) for k, v in tabs.items()}
        return tabs

    bacc_mod.get_activation_tables = _gat
    try:
        nc.compile()
    finally:
        bacc_mod.get_activation_tables = _orig_gat
    return nc


def _rope_tables():
    scales = 1.0 / (MAX_POS ** (np.arange(0, DH, 2, dtype=np.float32) / DH))
    freqs = np.outer(np.arange(S, dtype=np.float32), scales)  # [S, 32]
    cos = np.cos(freqs).T  # [32, S]
    sin = np.sin(freqs).T
    cos_dup = np.concatenate([cos, cos], axis=0)  # [64, S]
    sinswap = np.concatenate([sin, -sin], axis=0)  # [64, S]
    cos_t = np.concatenate([cos_dup, cos_dup], axis=0)  # [128, S] (2 heads)
    sin_t = np.concatenate([sinswap, sinswap], axis=0)
    return cos_t, sin_t


def _prep_inputs(x, kv_mask, attn_bias, W_qkv, b_qkv, W_out, b_out):
    scale = 1.0 / np.sqrt(DH)
    xT = np.ascontiguousarray(
        x.reshape(BS, D).T.astype(NPBF16)
    ).reshape(8, 128, BS)
    cos_t, sin_t = _rope_tables()
    cosq = (cos_t * scale).astype(NPBF16)
    sinq = (sin_t * scale).astype(NPBF16)
    cosk = cos_t.astype(NPBF16)
    sink = sin_t.astype(NPBF16)
    # mask vector [128, 32]: col = b*16 + sk_tile, row = position within tile
    mv = np.where(kv_mask, 0.0, NEG).astype(np.float32)  # [B, S]
    maskv = np.ascontiguousarray(
        mv.reshape(B, 16, 128).transpose(2, 0, 1).reshape(128, 32)
    )
    # bias: [b, q, k, h] -> [b, h, k, q] (bf16)
    bias_t = attn_bias.transpose(0, 3, 2, 1).astype(NPBF16)

    in_maps = []
    for c in range(NCORES):
        h0 = HPC * c
        def pmajor(w):
            # [1024, 128] -> [128 partitions, 8 kk-chunks, 128]
            return np.ascontiguousarray(
                w.astype(NPBF16).reshape(8, 128, 128).transpose(1, 0, 2)
            )

        wq = pmajor(W_qkv[:, h0 * DH:h0 * DH + 128])
        wk = pmajor(W_qkv[:, D + h0 * DH:D + h0 * DH + 128])
        wv = pmajor(W_qkv[:, 2 * D + h0 * DH:2 * D + h0 * DH + 128])
        wout = pmajor(W_out[:, c * 128:(c + 1) * 128])
        bias_c = np.ascontiguousarray(bias_t[:, h0:h0 + HPC])
        in_maps.append({
            "xT": xT, "wq": wq, "wk": wk, "wv": wv, "wout": wout,
            "cosq": cosq, "sinq": sinq, "cosk": cosk, "sink": sink,
            "maskv": maskv, "bias": bias_c,
        })
    return in_maps


def _run(inputs, trace=False):
    global _compiled
    if _compiled is None:
        _compiled = _build()
    in_maps = _prep_inputs(**inputs)
    res = run_bass_kernel_spmd(
        _compiled, in_maps, list(range(NCORES)), trace=trace
    )
    # each core returns outT [128, 4096]; transpose and concat on features
    cols = [res.results[c]["out"].T for c in range(NCORES)]
    out = np.concatenate(cols, axis=1).reshape(B, S, D)
    return out, res


def kernel(**inputs):
    out, _ = _run(inputs, trace=False)
    return out


# revision 42
# speedup vs baseline: 1.1068x; 1.0593x over previous
"""Distributed Trainium2 Bass kernel for nn_Attention_68736656605774.

Dense transformer self-attention block:
  qkv = x @ W_qkv + b_qkv ; RoPE(q, k) ; scores = q k^T/sqrt(dh) + mask + bias
  softmax ; a = P v ; out = a @ W_out + b_out

Sharding (8 cores): tensor-parallel over heads for qkv+attention (2 heads
per core, full batch), per-batch-half AllGather of the per-head attention
outputs (the first overlaps the second batch half's attention compute),
then column-parallel output projection (each core computes 128 of the 1024
output features; host concatenates).

Layout choices:
 - Everything head-side is feature-major ("transposed"): qT/kT are
   [feat, seq] so scores are computed directly transposed [Sk, Sq].  The
   kv-mask becomes a per-partition additive bias of the exp() activation,
   softmax needs no max-subtraction (logits are O(5)), and the softmax
   denominator comes for free from an all-ones column appended to v.
 - attn_bias is pre-transposed on host to [b, h, k, q] (bf16) and added
   to the scores in PSUM via PE identity matmuls.  Experiments that moved
   this add to the vector engine (multiplicative exp(bias)) were slower:
   with fewer than ~6 matmuls per score tile the tensor engine blocks on
   the exp pipeline every iteration and never holds its boosted clock
   p-state, which costs more than the identity matmuls save.
 - softmax normalization uses a_norm = a * exp(-ln(denom)); compilation
   is patched so the single activation table that holds both Exp and Ln
   serves every activation, eliminating per-head table reloads.
 - Collectives: batch 0 gathers whole (hidden under batch 1's attention);
   batch 1 gathers per query-half so the first half's output projection
   overlaps the final collective.  A collective trigger parks its issuing
   queue (gpsimd) until completion, so everything the attention loop
   needs rides the sync/scalar queues instead.
 - b_qkv / b_out are all-zero in this problem spec and are not applied.
"""

import sys

sys.path.insert(0, "/opt/trn_rl_repo")

import numpy as np
import ml_dtypes

import concourse.bass as bass
import concourse.mybir as mybir
import concourse.tile as tile
from concourse import bacc
from concourse.bass_utils import run_bass_kernel_spmd
from concourse.masks import make_identity

BF16 = mybir.dt.bfloat16
F32 = mybir.dt.float32
NPBF16 = ml_dtypes.bfloat16

NCORES = 8
B, S, D, H = 2, 2048, 1024, 16
DH = D // H  # 64
HPC = H // NCORES  # heads per core = 2
BS = B * S  # 4096
MAX_POS = 10000
NEG = -1e9
EXP = mybir.ActivationFunctionType.Exp
LN = mybir.ActivationFunctionType.Ln
ADD = mybir.AluOpType.add
MULT = mybir.AluOpType.mult

_compiled = None


def _build():
    nc = bacc.Bacc(None, num_devices=NCORES)

    xT_d = nc.declare_dram_parameter("xT", [8, 128, BS], BF16, isOutput=False)
    wq_d = nc.declare_dram_parameter("wq", [128, 8, 128], BF16, isOutput=False)
    wk_d = nc.declare_dram_parameter("wk", [128, 8, 128], BF16, isOutput=False)
    wv_d = nc.declare_dram_parameter("wv", [128, 8, 128], BF16, isOutput=False)
    wout_d = nc.declare_dram_parameter("wout", [128, 8, 128], BF16,
                                       isOutput=False)
    cosq_d = nc.declare_dram_parameter("cosq", [128, S], BF16, isOutput=False)
    sinq_d = nc.declare_dram_parameter("sinq", [128, S], BF16, isOutput=False)
    cosk_d = nc.declare_dram_parameter("cosk", [128, S], BF16, isOutput=False)
    sink_d = nc.declare_dram_parameter("sink", [128, S], BF16, isOutput=False)
    maskv_d = nc.declare_dram_parameter("maskv", [128, 32], F32, isOutput=False)
    bias_d = nc.declare_dram_parameter("bias", [B, HPC, S, S], BF16, isOutput=False)
    out_d = nc.declare_dram_parameter("out", [128, BS], F32, isOutput=True)

    with tile.TileContext(nc) as tc:
        with (
            tc.tile_pool(name="persist", bufs=1) as pp,
            tc.tile_pool(name="dram", bufs=1, space="DRAM") as dram,
        ):
            # ---------------- persistent SBUF tensors ----------------
            q_sb = pp.tile([128, BS], BF16, name="q_sb")
            k_sb = pp.tile([128, BS], BF16, name="k_sb")
            v_sb = pp.tile([128, 32, 130], BF16, name="v_sb")
            maskv = pp.tile([128, 32], F32, name="maskv")
            ones64 = pp.tile([1, 64], F32, name="ones64")
            ident = pp.tile([128, 128], BF16, name="ident")
            wout_sb = pp.tile([128, 8, 128], BF16, name="wout_sb")

            nc.vector.memset(ones64[:], 1.0)
            make_identity(nc, ident[:])
            # touch Exp once so the activation-table load lands here, in
            # the DMA shadow, instead of at the first attention tile
            scr1 = pp.tile([1, 1], F32, name="scr1")
            nc.scalar.activation(scr1[:], ones64[:, 0:1], EXP)

            # ---------------- phase 1: qkv projection + rope ----------------
            with (
                tc.tile_pool(name="ps1", bufs=8, space="PSUM") as ps1,
                tc.tile_pool(name="p1t", bufs=2) as p1t,
                tc.tile_pool(name="p1w", bufs=1) as p1w,
                tc.tile_pool(name="p1x", bufs=1) as p1x,
            ):
                xt_sb = p1x.tile([128, 8, BS], BF16, name="xt_sb")
                wq_sb = p1w.tile([128, 8, 128], BF16, name="wq_sb")
                wk_sb = p1w.tile([128, 8, 128], BF16, name="wk_sb")
                wv_sb = p1w.tile([128, 8, 128], BF16, name="wv_sb")
                cosq = p1w.tile([128, S], BF16, name="cosq")
                sinq = p1w.tile([128, S], BF16, name="sinq")
                cosk = p1w.tile([128, S], BF16, name="cosk")
                sink = p1w.tile([128, S], BF16, name="sink")
                # the kk-th chunk of xT is consumed ~3us after the
                # (kk-1)-th: split the 8 MB across two DMA queues, with all
                # the (small) stationary weights ahead of the bulky xT
                # chunks on sync, so the gemm never starves on either
                # operand.  Rope tables ride the otherwise-idle gpsimd
                # queue.
                for kk in range(0, 8, 2):
                    nc.scalar.dma_start(xt_sb[:, kk, :], xT_d[kk])
                nc.sync.dma_start(wq_sb[:], wq_d[:])
                for kk in range(1, 8, 2):
                    nc.sync.dma_start(xt_sb[:, kk, :], xT_d[kk])
                nc.sync.dma_start(wk_sb[:], wk_d[:])
                nc.sync.dma_start(wv_sb[:], wv_d[:])
                nc.sync.dma_start(maskv[:], maskv_d[:])
                nc.gpsimd.dma_start(cosq[:], cosq_d[:])
                nc.gpsimd.dma_start(sinq[:], sinq_d[:])
                nc.gpsimd.dma_start(cosk[:], cosk_d[:])
                nc.gpsimd.dma_start(sink[:], sink_d[:])

                qraw = p1w.tile([128, BS], BF16, name="qraw")
                kraw = p1w.tile([128, BS], BF16, name="kraw")
                vt_sb = p1w.tile([128, BS], BF16, name="vt_sb")

                # qT/kT/vT = W^T @ xT, feature-major [2*64, 4096];
                # kk-outer keeps the stationary operand loaded across the
                # 8 column chunks.  PSUM->SBUF copies ride on ACT, which
                # is otherwise idle in this phase.
                for w_sb, raw in ((wq_sb, qraw), (wk_sb, kraw), (wv_sb, vt_sb)):
                    pss = [
                        ps1.tile([128, 512], F32, name=f"ps_qk{n}", tag="ps1")
                        for n in range(8)
                    ]
                    for kk in range(8):
                        for n in range(8):
                            nc.tensor.matmul(
                                pss[n][:],
                                w_sb[:, kk, :],
                                xt_sb[:, kk, n * 512:(n + 1) * 512],
                                start=(kk == 0),
                                stop=(kk == 7),
                            )
                    for n in range(8):
                        nc.scalar.copy(
                            raw[:, n * 512:(n + 1) * 512], pss[n][:]
                        )

                # rope: q' = q*cos + swap32(q*sinswap), batch 0 first so
                # attention can begin while batch 1 is still rotating
                nc.vector.memset(v_sb[:, :, 64:65], 1.0)
                nc.vector.memset(v_sb[:, :, 129:130], 1.0)
                for b in range(B):
                    for raw, dst, ctab, stab in (
                        (qraw, q_sb, cosq, sinq),
                        (kraw, k_sb, cosk, sink),
                    ):
                        cols = slice(b * S, (b + 1) * S)
                        t = p1t.tile([128, S], BF16, name="rope_t", tag="rt")
                        m = p1t.tile([128, S], BF16, name="rope_m", tag="rm")
                        nc.vector.tensor_tensor(
                            t[:], raw[:, cols], ctab[:], MULT
                        )
                        # m[p] = raw[swap32(p)] * sinswap[swap32(p)]: shift
                        # partitions on the write side (both DVE read ports
                        # must share a base partition)
                        for blk in range(4):
                            p0 = blk * 32
                            sr = (blk ^ 1) * 32
                            nc.vector.tensor_tensor(
                                m[p0:p0 + 32, :],
                                raw[sr:sr + 32, cols],
                                stab[sr:sr + 32, :],
                                MULT,
                            )
                        nc.vector.tensor_tensor(
                            dst[:, cols], t[:], m[:], ADD
                        )
                    # v = transpose(vT) -> [seq, feat] tiles (ones columns
                    # at 64 / 129 for the softmax denominator)
                    for mt in range(b * 16, b * 16 + 16):
                        pst = ps1.tile([128, 128], BF16, name="ps_t",
                                       tag="ps1")
                        nc.tensor.transpose(
                            pst[:], vt_sb[:, mt * 128:(mt + 1) * 128],
                            ident[:]
                        )
                        nc.scalar.copy(
                            v_sb[:, mt, :].rearrange(
                                "p (h d) -> p h d", h=2
                            )[:, :, 0:64],
                            pst[:].rearrange("p (h d) -> p h d", h=2),
                        )

            # wout is only needed for the output projection; load it on
            # the quiet gpsimd queue
            nc.gpsimd.dma_start(wout_sb[:], wout_d[:])

            # ---------------- phase 2: attention ----------------
            p4a = tc.alloc_tile_pool(name="p4a", bufs=1)
            af_sb = p4a.tile([128, 8, BS], BF16, name="af_sb")
            # chunks: batch 0 gathers whole (its flight hides under
            # batch 1's attention); batch 1 gathers per query-half so the
            # first half's output projection overlaps the last collective
            ag_in = [
                dram.tile([128, S], BF16, name="ag_in0"),
                dram.tile([128, 1024], BF16, name="ag_in1a"),
                dram.tile([128, 1024], BF16, name="ag_in1b"),
            ]
            ag_out = [
                dram.tile([D, S], BF16, addr_space="Shared", name="ag_out0"),
                dram.tile([D, 1024], BF16, addr_space="Shared",
                          name="ag_out1a"),
                dram.tile([D, 1024], BF16, addr_space="Shared",
                          name="ag_out1b"),
            ]
            with (
                tc.tile_pool(name="ps_s", bufs=3, space="PSUM") as ps_sp,
                tc.tile_pool(name="ps_av", bufs=1, space="PSUM") as ps_avp,
                tc.tile_pool(name="p2t", bufs=8) as p2t,
                tc.tile_pool(name="p2s", bufs=8) as p2s,
                tc.tile_pool(name="p2n", bufs=2) as p2n,
            ):
                def emit_norm_b(state):
                    # part B of softmax normalize: broadcast -ln(denom) via
                    # PE, exponentiate, scale, ship to the allgather bounce
                    # buffer, and launch the batch collective when its last
                    # head lands
                    u_sb, ln_sb, bb, hroww, pww, fire_chunk = state
                    ps_bc = ps_sp.tile([64, 1024], F32, name="ps_bc", tag="s")
                    for j in range(2):
                        nc.tensor.matmul(
                            ps_bc[:, j * 512:(j + 1) * 512],
                            ones64[:],
                            ln_sb[:, j * 512:(j + 1) * 512],
                            start=True,
                            stop=True,
                        )
                    einv = p2n.tile([64, 1024], BF16, name="einv", tag="einv")
                    nc.scalar.activation(einv[:], ps_bc[:], EXP, scale=-1.0)
                    a_sb = p2n.tile([64, 1024], BF16, name="a_sb", tag="a")
                    nc.vector.tensor_tensor(
                        a_sb[:], u_sb[0:64, :], einv[:], MULT
                    )
                    if bb == 0:
                        nc.sync.dma_start(
                            ag_in[0][hroww, pww * 1024:(pww + 1) * 1024],
                            a_sb[:],
                        )
                    else:
                        nc.sync.dma_start(
                            ag_in[1 + pww][hroww, :], a_sb[:]
                        )
                    if fire_chunk is not None:
                        nc.gpsimd.collective_compute(
                            "AllGather",
                            mybir.AluOpType.bypass,
                            replica_groups=[list(range(NCORES))],
                            ins=[ag_in[fire_chunk].opt()],
                            outs=[ag_out[fire_chunk].opt()],
                        )

                pending_norm = None
                for b in range(B):
                    for pw in range(2):
                        q0 = b * S + pw * 1024
                        for h in range(HPC):
                            hrow = slice(h * 64, (h + 1) * 64)
                            vcols = slice(65 * h, 65 * h + 65)
                            ps_av = ps_avp.tile([65, 1024], F32,
                                                name="ps_av", tag="av")
                            pipe = []  # software pipeline: PV lags 2 tiles
                            for sk in range(16):
                                tg = b * 16 + sk
                                krows = slice(b * S + sk * 128,
                                              b * S + (sk + 1) * 128)
                                bias_sb = p2t.tile([128, 1024], BF16,
                                                   name="bias_sb", tag="bias")
                                nc.sync.dma_start(
                                    bias_sb[:],
                                    bias_d[b, h, sk * 128:(sk + 1) * 128,
                                           pw * 1024:(pw + 1) * 1024],
                                )
                                ps_s = ps_sp.tile([128, 1024], F32,
                                                  name="ps_s", tag="s")
                                for j in range(2):
                                    nc.tensor.matmul(
                                        ps_s[:, j * 512:(j + 1) * 512],
                                        k_sb[hrow, krows],
                                        q_sb[hrow, q0 + j * 512:
                                             q0 + (j + 1) * 512],
                                        start=True,
                                        stop=False,
                                    )
                                # bias via PE identity matmuls: keeps the
                                # tensor engine the clear pipeline
                                # bottleneck (it only holds the boosted
                                # clock when it never blocks on upstream
                                # semaphores) and keeps the exp dependency
                                # chain entirely on-PE
                                for j in range(2):
                                    nc.tensor.matmul(
                                        ps_s[:, j * 512:(j + 1) * 512],
                                        ident[:],
                                        bias_sb[:, j * 512:(j + 1) * 512],
                                        start=False,
                                        stop=True,
                                    )
                                exp_sb = p2s.tile([128, 1024], BF16,
                                                  name="exp_sb", tag="es")
                                nc.scalar.activation(
                                    exp_sb[:], ps_s[:], EXP,
                                    bias=maskv[:, tg:tg + 1], scale=1.0,
                                )
                                if sk == 2 and pending_norm is not None:
                                    emit_norm_b(pending_norm)
                                    pending_norm = None
                                if b == 1 and pw == 1 and h == 0 \
                                        and sk == 4:
                                    # prefetch batch 0's gathered heads on
                                    # gpsimd, emitted after the b1a
                                    # collective trigger: the queue is
                                    # parked there anyway, and a waiting
                                    # DMA must never sit ahead of the bias
                                    # stream (the sync queue runs ~8 tiles
                                    # ahead; one in-queue wait starves the
                                    # attention pipeline)
                                    for kk in range(8):
                                        nc.gpsimd.dma_start(
                                            af_sb[:, kk, 0:S],
                                            ag_out[0][kk * 128:
                                                      (kk + 1) * 128, :],
                                        )
                                pipe.append((tg, exp_sb))
                                if len(pipe) > 2:
                                    ptg, pp_sb = pipe.pop(0)
                                    for j in range(2):
                                        nc.tensor.matmul(
                                            ps_av[:, j * 512:(j + 1) * 512],
                                            v_sb[:, ptg, vcols],
                                            pp_sb[:, j * 512:(j + 1) * 512],
                                            start=(ptg % 16 == 0),
                                            stop=False,
                                        )
                            for di, (ptg, pp_sb) in enumerate(pipe):
                                last = di == len(pipe) - 1
                                for j in range(2):
                                    nc.tensor.matmul(
                                        ps_av[:, j * 512:(j + 1) * 512],
                                        v_sb[:, ptg, vcols],
                                        pp_sb[:, j * 512:(j + 1) * 512],
                                        start=False,
                                        stop=last,
                                    )
                            # normalize part A: ps_av -> SBUF (DVE) +
                            # ln(denom) (ACT); part B deferred to the next
                            # head's sk==2 so the PE never waits on it
                            u_sb = p2n.tile([65, 1024], F32, name="u_sb",
                                            tag="u")
                            nc.vector.tensor_scalar_mul(
                                u_sb[:], ps_av[:], 1.0
                            )
                            ln_sb = p2n.tile([1, 1024], F32, name="ln_sb",
                                             tag="ln")
                            nc.scalar.activation(ln_sb[:], u_sb[64:65, :], LN)
                            if h == HPC - 1 and (b == 1 or pw == 1):
                                fire = 0 if b == 0 else 1 + pw
                            else:
                                fire = None
                            pending_norm = (u_sb, ln_sb, b, hrow, pw, fire)
                if pending_norm is not None:
                    emit_norm_b(pending_norm)
                    pending_norm = None

            # ---------------- phase 4: output projection ----------------
            # column-parallel: this core computes output features
            # c*128..c*128+128 (its W_out column slice), transposed:
            # outT = Wc^T @ a_full^T.  Batch 0's chain only depends on the
            # first allgather (already landed), so it runs while the
            # second collective is still in flight; batch 1's matmuls
            # pipeline per-kk behind its af loads.
            with (
                tc.tile_pool(name="ps_o", bufs=8, space="PSUM") as ps_op,
                tc.tile_pool(name="p4t", bufs=4) as p4t,
            ):
                for c, (col0, src_ag) in enumerate(
                    ((0, None), (S, ag_out[1]), (S + 1024, ag_out[2]))
                ):
                    if src_ag is not None:
                        # b1a rides sync: the gpsimd queue is parked on the
                        # final collective's completion wait
                        ldq = nc.sync if c == 1 else nc.gpsimd
                        for kk in range(8):
                            ldq.dma_start(
                                af_sb[:, kk, col0:col0 + 1024],
                                src_ag[kk * 128:(kk + 1) * 128, :],
                            )
                    nsub = 4 if c == 0 else 2
                    ps_o = [
                        ps_op.tile([128, 512], F32, name=f"ps_o{c}{n}",
                                   tag="o")
                        for n in range(nsub)
                    ]
                    for kk in range(8):
                        for n in range(nsub):
                            nc.tensor.matmul(
                                ps_o[n][:],
                                wout_sb[:, kk, :],
                                af_sb[:, kk, col0 + n * 512:
                                      col0 + (n + 1) * 512],
                                start=(kk == 0),
                                stop=(kk == 7),
                            )
                    for n in range(nsub):
                        o_sb = p4t.tile([128, 512], F32, name="o_sb",
                                        tag="os")
                        nc.vector.tensor_scalar_mul(o_sb[:], ps_o[n][:], 1.0)
                        nc.scalar.dma_start(
                            out_d[:, col0 + n * 512:col0 + (n + 1) * 512],
                            o_sb[:],
                        )
            p4a.release()

    # compile with every activation served by the one table that holds
    # both Exp and Ln, so the per-head Ln never evicts the Exp table
    import concourse.bacc as bacc_mod
    _orig_gat = bacc_mod.get_activation_tables

    def _gat(arch):
        tabs = _orig_gat(arch)
        name = "natural_log_exp_and_others"
        if name in tabs:
            return {k: (v if k == name else set()) for k, v in tabs.items()}
        return tabs

    bacc_mod.get_activation_tables = _gat
    try:
        nc.compile()
    finally:
        bacc_mod.get_activation_tables = _orig_gat
    return nc


def _rope_tables():
    scales = 1.0 / (MAX_POS ** (np.arange(0, DH, 2, dtype=np.float32) / DH))
    freqs = np.outer(np.arange(S, dtype=np.float32), scales)  # [S, 32]
    cos = np.cos(freqs).T  # [32, S]
    sin = np.sin(freqs).T
    cos_dup = np.concatenate([cos, cos], axis=0)  # [64, S]
    sinswap = np.concatenate([sin, -sin], axis=0)  # [64, S]
    cos_t = np.concatenate([cos_dup, cos_dup], axis=0)  # [128, S] (2 heads)
    sin_t = np.concatenate([sinswap, sinswap], axis=0)
    return cos_t, sin_t


def _prep_inputs(x, kv_mask, attn_bias, W_qkv, b_qkv, W_out, b_out):
    scale = 1.0 / np.sqrt(DH)
    xT = np.ascontiguousarray(
        x.reshape(BS, D).T.astype(NPBF16)
    ).reshape(8, 128, BS)
    cos_t, sin_t = _rope_tables()
    cosq = (cos_t * scale).astype(NPBF16)
    sinq = (sin_t * scale).astype(NPBF16)
    cosk = cos_t.astype(NPBF16)
    sink = sin_t.astype(NPBF16)
    # mask vector [128, 32]: col = b*16 + sk_tile, row = position within tile
    mv = np.where(kv_mask, 0.0, NEG).astype(np.float32)  # [B, S]
    maskv = np.ascontiguousarray(
        mv.reshape(B, 16, 128).transpose(2, 0, 1).reshape(128, 32)
    )
    # bias: [b, q, k, h] -> [b, h, k, q] (bf16)
    bias_t = attn_bias.transpose(0, 3, 2, 1).astype(NPBF16)

    in_maps = []
    for c in range(NCORES):
        h0 = HPC * c
        def pmajor(w):
            # [1024, 128] -> [128 partitions, 8 kk-chunks, 128]
            return np.ascontiguousarray(
                w.astype(NPBF16).reshape(8, 128, 128).transpose(1, 0, 2)
            )

        wq = pmajor(W_qkv[:, h0 * DH:h0 * DH + 128])
        wk = pmajor(W_qkv[:, D + h0 * DH:D + h0 * DH + 128])
        wv = pmajor(W_qkv[:, 2 * D + h0 * DH:2 * D + h0 * DH + 128])
        wout = pmajor(W_out[:, c * 128:(c + 1) * 128])
        bias_c = np.ascontiguousarray(bias_t[:, h0:h0 + HPC])
        in_maps.append({
            "xT": xT, "wq": wq, "wk": wk, "wv": wv, "wout": wout,
            "cosq": cosq, "sinq": sinq, "cosk": cosk, "sink": sink,
            "maskv": maskv, "bias": bias_c,
        })
    return in_maps


def _run(inputs, trace=False):
    global _compiled
    if _compiled is None:
        _compiled = _build()
    in_maps = _prep_inputs(**inputs)
    res = run_bass_kernel_spmd(
        _compiled, in_maps, list(range(NCORES)), trace=trace
    )
    # each core returns outT [128, 4096]; transpose and concat on features
    cols = [res.results[c]["out"].T for c in range(NCORES)]
    out = np.concatenate(cols, axis=1).reshape(B, S, D)
    return out, res


def kernel(**inputs):
    out, _ = _run(inputs, trace=False)
    return out
